# revision 8
# baseline (speedup 1.0000x reference)
"""MoE routing layer kernel for 8 Trainium2 NeuronCores.

Math (matching the reference exactly):
  logits  = x @ gate_w.T + gate_b + expert_biases            [BS, 8]
  probs   = sigmoid(logits); top2 by value (ties -> lower index)
  proj_j[t, e] = x[t] . expert_w[e, j, :] + expert_b[e, j]   (j = 0, 1)
  val_j   = proj_j[t, idx_j]
  out[t, :] = (val_0 * p_0 + val_1 * p_1) / (p_0 + p_1)      broadcast over 1024
Only rows 0..K-1 of each expert's weight matrix are ever used (the reference
gathers with the top-k slot as the feature index), so the device computes a
single fused 24-column matmul: [gate(8) | expert row0 (8) | expert row1 (8)].

Sharding: data-parallel over the 8192 tokens, 1024 tokens per core; the
24x1024 fused weight matrix and small constants are replicated.
"""

import numpy as np

import concourse.bass as bass
import concourse.bacc as bacc
import concourse.mybir as mybir
from concourse import bass_utils
from concourse.tile import TileContext

# Problem shape (hardcoded per contract).
B, S, D, E, K = 4, 2048, 1024, 8, 2
N_CORES = 8
TOK = B * S                   # 8192 tokens total
TPC = TOK // N_CORES          # 1024 tokens per core
P = 128                       # partitions
NT = TPC // P                 # 8 token tiles per core
DC = D // P                   # 8 contraction chunks
W24 = 3 * E                   # 24 fused output columns

F32 = mybir.dt.float32
I32 = mybir.dt.int32
U32 = mybir.dt.uint32
AX = mybir.AxisListType
ALU = mybir.AluOpType
ACTF = mybir.ActivationFunctionType


def build_kernel_body(nc, tc, ins, outs):
    xs, wt, brep, ident = ins["xs"], ins["wt"], ins["brep"], ins["ident"]
    out, idx = outs["out"], outs["idx"]

    from contextlib import ExitStack
    with ExitStack() as ctx:
        # bufs=NT on SBUF pools: no slot reuse within the kernel, so DMA and
        # PE instructions (1 ISA wait slot each) never pick up WAR/WAW waits.
        const = ctx.enter_context(tc.tile_pool(name="const", bufs=1))
        xpool = ctx.enter_context(tc.tile_pool(name="xnat", bufs=NT))
        xtpool = ctx.enter_context(tc.tile_pool(name="xT", bufs=NT))
        scps = ctx.enter_context(tc.tile_pool(name="scps", bufs=1, space="PSUM"))
        trps = ctx.enter_context(tc.tile_pool(name="trp", bufs=4, space="PSUM"))
        pjps = ctx.enter_context(tc.tile_pool(name="pjp", bufs=2, space="PSUM"))
        spool = ctx.enter_context(tc.tile_pool(name="small", bufs=NT))
        opool = ctx.enter_context(tc.tile_pool(name="obuf", bufs=NT))

        wt_s = const.tile([P, DC * W24], F32, tag="wt")
        brep_s = const.tile([P, W24], F32, tag="brep")
        ident_s = const.tile([P, P], F32, tag="ident")
        nc.sync.dma_start(wt_s, wt)
        nc.sync.dma_start(brep_s, brep)
        nc.sync.dma_start(ident_s, ident)

        # PE instructions (LDW) carry at most ONE sync wait in the ISA, so
        # absorb each const-DMA dependency into its own dummy PE op up front;
        # real PE work then only ever waits on one new semaphore at a time.
        scratch = scps.tile([P, P], F32, tag="scratch")
        nc.tensor.transpose(scratch, ident_s, ident_s)
        nc.tensor.matmul(scratch[0:W24, 0:1], wt_s[:, 0:W24], ident_s[:, 0:1],
                         start=True, stop=True, skip_group_check=True)
        # same trick for the first DVE consumer of brep
        btouch = const.tile([P, 1], F32, tag="btouch")
        nc.vector.tensor_copy(btouch, brep_s[:, 0:1])

        for t in range(NT):
            rows = slice(t * P, (t + 1) * P)
            x_nat = xpool.tile([P, D], F32, tag="xnat")
            nc.sync.dma_start(x_nat, xs[rows, :])

            # x^T chunks via PE transpose (fp32), PSUM -> SBUF copy on DVE
            xT = xtpool.tile([P, D], F32, tag="xT")
            for c in range(DC):
                cols = slice(c * P, (c + 1) * P)
                trp = trps.tile([P, P], F32, tag="trp")
                nc.tensor.transpose(trp, x_nat[:, cols], ident_s)
                nc.vector.tensor_copy(xT[:, cols], trp)

            # fused 24-wide projection, accumulated over the 8 D-chunks
            pj = pjps.tile([P, W24], F32, tag="pj")
            for c in range(DC):
                nc.tensor.matmul(
                    pj,
                    xT[:, c * P:(c + 1) * P],
                    wt_s[:, c * W24:(c + 1) * W24],
                    start=(c == 0),
                    stop=(c == DC - 1),
                )
            proj = spool.tile([P, W24], F32, tag="proj")
            nc.vector.tensor_add(proj, pj, brep_s)

            # top-2 over the 8 gate logits (monotone in probs)
            mx = spool.tile([P, 8], F32, tag="mx")
            mi = spool.tile([P, 8], U32, tag="mi")
            nc.vector.max(out=mx, in_=proj[:, 0:E])
            nc.vector.max_index(out=mi, in_max=mx, in_values=proj[:, 0:E])

            # probabilities of the two selected experts
            p12 = spool.tile([P, 2], F32, tag="p12")
            nc.scalar.activation(p12, mx[:, 0:2], ACTF.Sigmoid)

            # one-hot masks over experts by exact logit match
            msk = spool.tile([P, 2 * E], F32, tag="msk")
            nc.vector.tensor_tensor(
                msk[:, 0:E], proj[:, 0:E],
                mx[:, 0:1].to_broadcast([P, E]), op=ALU.is_equal)
            nc.vector.tensor_tensor(
                msk[:, E:2 * E], proj[:, 0:E],
                mx[:, 1:2].to_broadcast([P, E]), op=ALU.is_equal)

            # val_j = sum_e proj_j[:, e] * mask_j[:, e]
            vv = spool.tile([P, 2 * E], F32, tag="vv")
            nc.vector.tensor_mul(vv, proj[:, E:3 * E], msk)
            val = spool.tile([P, 2], F32, tag="val")
            nc.vector.reduce_sum(
                val, vv.rearrange("p (j e) -> p j e", j=2), axis=AX.X)

            # weighted = (val0*p0 + val1*p1) / (p0 + p1)
            wv = spool.tile([P, 2], F32, tag="wv")
            nc.vector.tensor_mul(wv, val, p12)
            num = spool.tile([P, 1], F32, tag="num")
            nc.vector.reduce_sum(num, wv, axis=AX.X)
            den = spool.tile([P, 1], F32, tag="den")
            nc.vector.reduce_sum(den, p12, axis=AX.X)
            rden = spool.tile([P, 1], F32, tag="rden")
            nc.vector.reciprocal(rden, den)
            wgt = spool.tile([P, 1], F32, tag="wgt")
            nc.vector.tensor_mul(wgt, num, rden)

            # broadcast across the 1024 output features and store
            obuf = opool.tile([P, D], F32, tag="obuf")
            nc.scalar.activation(obuf, wgt.to_broadcast([P, D]), ACTF.Copy)
            nc.gpsimd.dma_start(out[rows, :], obuf)

            ii = spool.tile([P, 2], I32, tag="ii")
            nc.vector.tensor_copy(ii, mi[:, 0:2])
            nc.gpsimd.dma_start(idx[rows, :], ii)


def _prep_shared(gate_w, gate_b, expert_biases, expert_w, expert_b):
    """Host-side packing of the replicated small tensors."""
    w24 = np.concatenate(
        [gate_w, expert_w[:, 0, :], expert_w[:, 1, :]], axis=0)  # [24, 1024]
    # wt[p, c*24 + j] = w24[j, c*128 + p]
    wt = np.ascontiguousarray(
        w24.T.reshape(DC, P, W24).transpose(1, 0, 2).reshape(P, DC * W24))
    b24 = np.concatenate(
        [gate_b + expert_biases, expert_b[:, 0], expert_b[:, 1]])  # [24]
    brep = np.ascontiguousarray(
        np.broadcast_to(b24.astype(np.float32), (P, W24)))
    ident = np.eye(P, dtype=np.float32)
    return wt.astype(np.float32), brep, ident


def _build_module():
    nc = bacc.Bacc("TRN2", target_bir_lowering=False, debug=False,
                   num_devices=N_CORES)
    ins = {
        "xs": nc.dram_tensor("xs", [TPC, D], F32, kind="ExternalInput").ap(),
        "wt": nc.dram_tensor("wt", [P, DC * W24], F32, kind="ExternalInput").ap(),
        "brep": nc.dram_tensor("brep", [P, W24], F32, kind="ExternalInput").ap(),
        "ident": nc.dram_tensor("ident", [P, P], F32, kind="ExternalInput").ap(),
    }
    outs = {
        "out": nc.dram_tensor("out", [TPC, D], F32, kind="ExternalOutput").ap(),
        "idx": nc.dram_tensor("idx", [TPC, 2], I32, kind="ExternalOutput").ap(),
    }
    with TileContext(nc) as tc:
        build_kernel_body(nc, tc, ins, outs)
    nc.compile()
    return nc


_NC_CACHE = None


def _get_module():
    global _NC_CACHE
    if _NC_CACHE is None:
        _NC_CACHE = _build_module()
    return _NC_CACHE


def _run(inputs, trace=False, trace_kwargs=None):
    x = np.asarray(inputs["x"], np.float32)
    wt, brep, ident = _prep_shared(
        np.asarray(inputs["gate_w"], np.float32),
        np.asarray(inputs["gate_b"], np.float32),
        np.asarray(inputs["expert_biases"], np.float32),
        np.asarray(inputs["expert_w"], np.float32),
        np.asarray(inputs["expert_b"], np.float32),
    )
    xf = x.reshape(TOK, D)
    shared = {"wt": wt, "brep": brep, "ident": ident}
    in_maps = [
        {"xs": np.ascontiguousarray(xf[c * TPC:(c + 1) * TPC, :]), **shared}
        for c in range(N_CORES)
    ]
    nc = _get_module()
    kw = {}
    if trace:
        kw["trace"] = True
        kw["trace_cores"] = list(range(N_CORES))
        if trace_kwargs:
            kw["trace_kwargs"] = trace_kwargs
    res = bass_utils.run_bass_kernel_spmd(
        nc, in_maps, core_ids=list(range(N_CORES)), **kw)
    out = np.concatenate([res.results[c]["out"] for c in range(N_CORES)],
                         axis=0).reshape(B, S, D)
    idx = np.concatenate([res.results[c]["idx"] for c in range(N_CORES)],
                         axis=0).reshape(B, S, K)
    return (out.astype(np.float32), idx.astype(np.int32)), res


def kernel(**inputs):
    (out, idx), _ = _run(inputs)
    return out, idx
